# revision 12
# baseline (speedup 1.0000x reference)
"""Trainium2 Bass kernel for a 4-layer GPT (B=2, T=1024, D=768, H=12, V=32000).

Sharding: 8 cores = 2 (batch) x 4 (token chunks of 256).  Each layer:
token-local qkv -> AllGather(K,V) within the 4-core batch group -> full
causal attention for own queries -> proj+residual -> MLP (token-local).
Final: 8-core AllGather of hidden states, vocab-sharded tied lm_head
(4096 padded-vocab rows per core).  Host: embedding gather, LN folding
into adjacent matmul weights, bf16 weight casting, logits assembly and
cross-entropy loss.

Self-contained: hardcodes all shapes from the problem spec.
"""

import sys

sys.path.insert(0, "/opt/trn_rl_repo")

import numpy as np
import ml_dtypes

import concourse.bass as bass
import concourse.mybir as mybir
import concourse.tile as tile
from concourse import bacc, bass_utils
from concourse.masks import make_identity

P = 128
D = 768
DO = D // P          # 6
H = 12
HD = 64
TOK = 256            # tokens per core
NSH = 4              # shards (cores) per batch group
KCH = NSH * TOK // P # 8 key chunks of 128
QKVO = 3 * D // P    # 18
F1O = 4 * D // P     # 24
NCORES = 8

FULL_CFG = dict(L=4, VSH=4096, B=2, T=1024, V=32000)

F32 = mybir.dt.float32
BF16 = mybir.dt.bfloat16


def build_nc(cfg, weights):
    """weights: list of per-layer dicts (wqkv/wproj/wfc1/wfc2 bf16, bias f32)
    identical on all cores -- embedded in the NEFF as Const tensors."""
    L = cfg["L"]
    VSH = cfg["VSH"]
    VO = VSH // P          # vocab chunks per core
    VSUB = VSH // 512      # streamed wte subtiles

    nc = bacc.Bacc("TRN2", target_bir_lowering=False, debug=False,
                   num_devices=NCORES)

    # ---- DRAM I/O ----
    x0t_d = nc.dram_tensor("x0t", [D, TOK], F32, kind="ExternalInput")
    mask_d = nc.dram_tensor("mask01t", [NSH * TOK, TOK], BF16,
                            kind="ExternalInput")
    w_d = []
    for l in range(L):
        w_d.append({k: nc.inline_tensor(weights[l][k], name=f"{k}_{l}")
                    for k in ("wqkv", "wproj", "wfc1", "wfc2", "bias")})
    wtet_d = nc.dram_tensor("wtet", [D, VSH], BF16, kind="ExternalInput")
    bhead_d = nc.dram_tensor("bhead", [VSH], F32, kind="ExternalInput")
    logits_d = nc.dram_tensor("logitst", [VSH, 2048], F32, kind="ExternalOutput")

    KBYTES = D * TOK          # elements in k^T part of the kv exchange buffer
    KVLEN = 2 * KBYTES        # k^T then v(token-major), flat

    with tile.TileContext(nc) as tc:
        with (
            tc.tile_pool(name="const", bufs=1) as cpool,
            tc.tile_pool(name="state", bufs=1) as spool,
            tc.tile_pool(name="acts", bufs=1) as apool,
            tc.tile_pool(name="wts", bufs=3) as wpool,
            tc.tile_pool(name="bias", bufs=2) as bpool,
            tc.tile_pool(name="tmp", bufs=3) as tpool,
            tc.tile_pool(name="pmask", bufs=2) as ppool,
            tc.tile_pool(name="mmps", bufs=4, space="PSUM") as mmps,
            tc.tile_pool(name="dram", bufs=1, space="DRAM") as dpool,
        ):
            # constants
            ones_f = cpool.tile([P, P], F32, name="ones_f")
            nc.gpsimd.memset(ones_f[:], 1.0)
            ones_b = cpool.tile([P, 1], BF16, name="ones_b")
            nc.gpsimd.memset(ones_b[:], 1.0)
            ident = cpool.tile([P, P], BF16, name="ident")
            make_identity(nc, ident[:])
            eps_t = cpool.tile([1, 1], F32, name="eps_t")
            nc.gpsimd.memset(eps_t[:], 1e-5)
            maskt = cpool.tile([P, KCH, TOK], BF16, name="maskt")
            nc.sync.dma_start(
                maskt[:], mask_d.ap().rearrange("(kc kp) q -> kp kc q", kp=P))

            # residual stream, f32, feature-major [feat%128, feat//128, tok]
            x = spool.tile([P, DO, TOK], F32, name="x")
            nc.sync.dma_start(
                x[:], x0t_d.ap().rearrange("(o p) t -> p o t", p=P))

            def layer_norm(x_in, out_tile):
                """out = (x - mean)/std featurewise (weights folded on host)."""
                sq = tpool.tile([P, DO, TOK], F32, name="sq", tag="sq", bufs=1)
                nc.vector.tensor_tensor(sq[:], x_in[:], x_in[:],
                                        mybir.AluOpType.mult)
                s_ps = mmps.tile([P, 512], F32, name="s_ps", tag="mm")
                q_ps = mmps.tile([P, 512], F32, name="q_ps", tag="mm")
                for ko in range(DO):
                    nc.tensor.matmul(s_ps[0:1, 0:TOK], ones_f[:, 0:1],
                                     x_in[:, ko, :],
                                     start=(ko == 0), stop=(ko == DO - 1))
                    nc.tensor.matmul(q_ps[0:1, 0:TOK], ones_f[:, 0:1],
                                     sq[:, ko, :],
                                     start=(ko == 0), stop=(ko == DO - 1))
                mr = tpool.tile([1, 2 * TOK], F32, name="mr", tag="mr", bufs=2)
                # mean into [0:TOK]
                nc.scalar.activation(mr[0:1, 0:TOK], s_ps[0:1, 0:TOK],
                                     mybir.ActivationFunctionType.Copy,
                                     scale=1.0 / D)
                # E[x^2]
                ex2 = tpool.tile([1, TOK], F32, name="ex2", tag="ex2", bufs=2)
                nc.scalar.activation(ex2[0:1, :], q_ps[0:1, 0:TOK],
                                     mybir.ActivationFunctionType.Copy,
                                     scale=1.0 / D)
                # var = ex2 - mean^2, via (mean * mean) then subtract
                var = tpool.tile([1, TOK], F32, name="var", tag="var", bufs=2)
                nc.vector.tensor_tensor(var[0:1, :], mr[0:1, 0:TOK],
                                        mr[0:1, 0:TOK], mybir.AluOpType.mult)
                nc.vector.tensor_tensor(var[0:1, :], ex2[0:1, :], var[0:1, :],
                                        mybir.AluOpType.subtract)
                sd = tpool.tile([1, TOK], F32, name="sd", tag="sd", bufs=2)
                nc.scalar.activation(sd[0:1, :], var[0:1, :],
                                     mybir.ActivationFunctionType.Sqrt,
                                     bias=eps_t[0:1, 0:1])
                nc.vector.reciprocal(mr[0:1, TOK:2 * TOK], sd[0:1, :])
                b_ps = mmps.tile([P, 512], F32, name="b_ps", tag="mm")
                nc.tensor.matmul(b_ps[:, 0:2 * TOK], ones_f[0:1, :],
                                 mr[0:1, :], start=True, stop=True)
                for ko in range(DO):
                    t1 = tpool.tile([P, TOK], F32, name="lnt", tag="lnt", bufs=3)
                    nc.vector.tensor_tensor(t1[:], x_in[:, ko, :],
                                            b_ps[:, 0:TOK],
                                            mybir.AluOpType.subtract)
                    nc.vector.tensor_tensor(out_tile[:, ko, :], t1[:],
                                            b_ps[:, TOK:2 * TOK],
                                            mybir.AluOpType.mult)

            for l in range(L):
                wd = w_d[l]
                bias_t = bpool.tile([P, 54], F32, name=f"bias_t{l}", tag="bias")
                nc.sync.dma_start(
                    bias_t[:], wd["bias"].ap().rearrange("(o p) -> p o", p=P))
                bq = bias_t[:, 0:QKVO]
                bp = bias_t[:, QKVO:QKVO + DO]
                b1 = bias_t[:, QKVO + DO:QKVO + DO + F1O]
                b2 = bias_t[:, QKVO + DO + F1O:54]

                # ---- LN1 + qkv ----
                h = apool.tile([P, DO, TOK], BF16, name="h", tag="h")
                layer_norm(x, h)
                qkvt = apool.tile([P, QKVO, TOK], BF16, name="qkvt", tag="qkvt")
                wq_view = wd["wqkv"].ap().rearrange("(ko kp) m -> kp ko m", kp=P)
                for ms in range(3):
                    wsub = wpool.tile([P, DO, D], BF16, name=f"wq{l}_{ms}",
                                      tag="wsub")
                    nc.sync.dma_start(wsub[:],
                                      wq_view[:, :, ms * D:(ms + 1) * D])
                    for mo in range(DO):
                        ps = mmps.tile([P, 512], F32, name="qkv_ps", tag="mm")
                        for ko in range(DO):
                            nc.tensor.matmul(
                                ps[:, 0:TOK],
                                wsub[:, ko, mo * P:(mo + 1) * P],
                                h[:, ko, :],
                                start=(ko == 0), stop=(ko == DO - 1))
                        gm = 6 * ms + mo
                        nc.scalar.activation(
                            qkvt[:, gm, :], ps[:, 0:TOK],
                            mybir.ActivationFunctionType.Identity,
                            bias=bq[:, gm:gm + 1])

                # ---- local v -> token-major ----
                vtok = apool.tile([P, 2, D], BF16, name="vtok", tag="vtok")
                with tc.tile_pool(name="vtps", bufs=2, space="PSUM") as vtps:
                    for o in range(DO):
                        for to in range(2):
                            tp_ps = vtps.tile([P, P], BF16, name="tp_ps")
                            nc.tensor.transpose(
                                tp_ps[:], qkvt[:, 12 + o, to * P:(to + 1) * P],
                                ident[:])
                            nc.scalar.activation(
                                vtok[:, to, o * P:(o + 1) * P], tp_ps[:],
                                mybir.ActivationFunctionType.Copy)

                # ---- kv exchange (AllGather within batch group) ----
                kv_in = dpool.tile([KVLEN], BF16, name=f"kv_in{l}",
                                   tag="kv_in", bufs=2)
                nc.sync.dma_start(
                    kv_in[0:KBYTES].rearrange("(o p t) -> p o t", p=P, t=TOK),
                    qkvt[:, 6:12, :])
                nc.sync.dma_start(
                    kv_in[KBYTES:KVLEN].rearrange("(to tp d) -> tp to d",
                                                  tp=P, d=D),
                    vtok[:])
                kv_g = dpool.tile([NSH, KVLEN], BF16, name=f"kv_g{l}",
                                  tag="kv_g", bufs=2)
                nc.gpsimd.collective_compute(
                    "AllGather", mybir.AluOpType.bypass,
                    replica_groups=[[0, 1, 2, 3], [4, 5, 6, 7]],
                    ins=[kv_in[:]], outs=[kv_g[:]])

                kt = apool.tile([P, DO, NSH, TOK], BF16, name="kt", tag="kt")
                vt = apool.tile([P, NSH, 2, D], BF16, name="vt", tag="vt")
                for si in range(NSH):
                    nc.sync.dma_start(
                        kt[:, :, si, :],
                        kv_g[si, 0:KBYTES].rearrange("(o p t) -> p o t",
                                                     p=P, t=TOK))
                    nc.sync.dma_start(
                        vt[:, si, :, :],
                        kv_g[si, KBYTES:KVLEN].rearrange("(to tp d) -> tp to d",
                                                         tp=P, d=D))

                # ---- attention ----
                yt = apool.tile([P, DO, TOK], BF16, name="yt", tag="yt")
                with tc.tile_pool(name="attnps", bufs=1, space="PSUM") as atps:
                    for hh in range(H):
                        ko_h = hh // 2
                        p0 = HD * (hh % 2)
                        s_ps = atps.tile([P, KCH, TOK], F32, name="s_ps")
                        for kc in range(KCH):
                            si, th = kc // 2, kc % 2
                            nc.tensor.matmul(
                                s_ps[:, kc, :],
                                kt[p0:p0 + HD, ko_h, si, th * P:(th + 1) * P],
                                qkvt[p0:p0 + HD, ko_h, :],
                                start=True, stop=True)
                        pm = ppool.tile([P, KCH, TOK], BF16, name="pm",
                                        tag="pm")
                        nc.scalar.activation(
                            pm[:], s_ps[:],
                            mybir.ActivationFunctionType.Exp)
                        nc.vector.tensor_tensor(pm[:], pm[:], maskt[:],
                                                mybir.AluOpType.mult)
                        sum_ps = mmps.tile([P, 512], F32, name="sum_ps", tag="mm")
                        for kc in range(KCH):
                            nc.tensor.matmul(sum_ps[0:1, 0:TOK], ones_b[:],
                                             pm[:, kc, :],
                                             start=(kc == 0),
                                             stop=(kc == KCH - 1))
                        rcp = tpool.tile([1, TOK], F32, name="rcp", tag="rcp",
                                         bufs=2)
                        nc.vector.reciprocal(rcp[0:1, :], sum_ps[0:1, 0:TOK])
                        rb_ps = mmps.tile([P, 512], F32, name="rb_ps", tag="mm")
                        nc.tensor.matmul(rb_ps[:, 0:TOK], ones_f[0:1, :],
                                         rcp[0:1, :], start=True, stop=True)
                        rb = tpool.tile([P, TOK], F32, name="rb", tag="rb",
                                        bufs=2)
                        nc.scalar.activation(
                            rb[:], rb_ps[:, 0:TOK],
                            mybir.ActivationFunctionType.Copy)
                        av_ps = mmps.tile([P, 512], F32, name="av_ps", tag="mm")
                        for kc in range(KCH):
                            nc.tensor.matmul(
                                av_ps[0:HD, 0:TOK],
                                vt[:, kc // 2, kc % 2, hh * HD:(hh + 1) * HD],
                                pm[:, kc, :],
                                start=(kc == 0), stop=(kc == KCH - 1))
                        nc.vector.tensor_tensor(
                            yt[p0:p0 + HD, ko_h, :], av_ps[0:HD, 0:TOK],
                            rb[0:HD, :], mybir.AluOpType.mult)

                # ---- proj + residual ----
                wp = wpool.tile([P, DO, D], BF16, name=f"wp{l}", tag="wsub")
                nc.sync.dma_start(
                    wp[:], wd["wproj"].ap().rearrange("(ko kp) m -> kp ko m",
                                                      kp=P))
                for mo in range(DO):
                    ps = mmps.tile([P, 512], F32, name="proj_ps", tag="mm")
                    for ko in range(DO):
                        nc.tensor.matmul(ps[:, 0:TOK],
                                         wp[:, ko, mo * P:(mo + 1) * P],
                                         yt[:, ko, :],
                                         start=(ko == 0), stop=(ko == DO - 1))
                    nc.vector.scalar_tensor_tensor(
                        x[:, mo, :], ps[:, 0:TOK], bp[:, mo:mo + 1],
                        x[:, mo, :],
                        mybir.AluOpType.add, mybir.AluOpType.add)

                # ---- LN2 + MLP ----
                h2 = apool.tile([P, DO, TOK], BF16, name="h2", tag="h")
                layer_norm(x, h2)
                g = apool.tile([P, F1O, TOK], BF16, name="g", tag="g")
                w1_view = wd["wfc1"].ap().rearrange("(ko kp) m -> kp ko m", kp=P)
                for ms in range(4):
                    wsub = wpool.tile([P, DO, D], BF16, name=f"w1{l}_{ms}",
                                      tag="wsub")
                    nc.sync.dma_start(wsub[:],
                                      w1_view[:, :, ms * D:(ms + 1) * D])
                    for mo in range(DO):
                        ps = mmps.tile([P, 512], F32, name="fc1_ps", tag="mm")
                        for ko in range(DO):
                            nc.tensor.matmul(
                                ps[:, 0:TOK],
                                wsub[:, ko, mo * P:(mo + 1) * P],
                                h2[:, ko, :],
                                start=(ko == 0), stop=(ko == DO - 1))
                        gm = 6 * ms + mo
                        nc.scalar.activation(
                            g[:, gm, :], ps[:, 0:TOK],
                            mybir.ActivationFunctionType.Gelu,
                            bias=b1[:, gm:gm + 1])

                w2_view = wd["wfc2"].ap().rearrange("(ko kp) m -> kp ko m", kp=P)
                w2subs = []
                for ks in range(4):
                    wsub = wpool.tile([P, DO, D], BF16, name=f"w2{l}_{ks}",
                                      tag="w2sub", bufs=4)
                    nc.sync.dma_start(
                        wsub[:],
                        w2_view[:, ks * DO:(ks + 1) * DO, :])
                    w2subs.append(wsub)
                with tc.tile_pool(name="fc2ps", bufs=1, space="PSUM") as f2ps:
                    ps = f2ps.tile([P, DO, TOK], F32, name="fc2_ps")
                    for mo in range(DO):
                        for ks in range(4):
                            for ko in range(DO):
                                nc.tensor.matmul(
                                    ps[:, mo, :],
                                    w2subs[ks][:, ko, mo * P:(mo + 1) * P],
                                    g[:, 6 * ks + ko, :],
                                    start=(ks == 0 and ko == 0),
                                    stop=(ks == 3 and ko == DO - 1))
                    for mo in range(DO):
                        nc.vector.scalar_tensor_tensor(
                            x[:, mo, :], ps[:, mo, :], b2[:, mo:mo + 1],
                            x[:, mo, :],
                            mybir.AluOpType.add, mybir.AluOpType.add)

            # ---- final LN + hidden-state AllGather ----
            xhat = apool.tile([P, DO, TOK], BF16, name="xhat", tag="h")
            layer_norm(x, xhat)
            ag2_in = dpool.tile([D * TOK], BF16, name="ag2_in")
            nc.sync.dma_start(
                ag2_in[:].rearrange("(o p t) -> p o t", p=P, t=TOK), xhat[:])
            xall_d = dpool.tile([NCORES, D * TOK], BF16, name="xall_d",
                                addr_space="Shared")
            nc.gpsimd.collective_compute(
                "AllGather", mybir.AluOpType.bypass,
                replica_groups=[[0, 1, 2, 3, 4, 5, 6, 7]],
                ins=[ag2_in[:]], outs=[xall_d[:]])
            xall = apool.tile([P, DO, NCORES, TOK], BF16, name="xall")
            for si in range(NCORES):
                nc.sync.dma_start(
                    xall[:, :, si, :],
                    xall_d[si, :].rearrange("(o p t) -> p o t", p=P, t=TOK))

            # ---- lm head ----
            bh_t = cpool.tile([P, VO], F32, name="bh_t")
            nc.sync.dma_start(
                bh_t[:], bhead_d.ap().rearrange("(o p) -> p o", p=P))
            wte_view = wtet_d.ap().rearrange("(ko kp) v -> kp ko v", kp=P)
            lg_view = logits_d.ap().rearrange("(vc vp) t -> vp vc t", vp=P)
            for vs in range(VSUB):
                wt = wpool.tile([P, DO, 512], BF16, name=f"wt{vs}", tag="wsub")
                nc.sync.dma_start(wt[:],
                                  wte_view[:, :, vs * 512:(vs + 1) * 512])
                for vloc in range(4):
                    vc = 4 * vs + vloc
                    for tcb in range(4):
                        ps = mmps.tile([P, 512], F32, name="hd_ps", tag="mm")
                        for ko in range(DO):
                            nc.tensor.matmul(
                                ps[:],
                                wt[:, ko, vloc * P:(vloc + 1) * P],
                                xall[:, ko, 2 * tcb:2 * tcb + 2, :],
                                start=(ko == 0), stop=(ko == DO - 1))
                        lsb = tpool.tile([P, 512], F32, name="lsb", tag="lsb",
                                         bufs=4)
                        nc.scalar.activation(
                            lsb[:], ps[:],
                            mybir.ActivationFunctionType.Identity,
                            bias=bh_t[:, vc:vc + 1])
                        nc.sync.dma_start(
                            lg_view[:, vc, tcb * 512:(tcb + 1) * 512], lsb[:])

    nc.compile()
    return nc


# ----------------------------------------------------------------------
# Host-side preparation
# ----------------------------------------------------------------------

def fold_weights(inputs, cfg):
    """Fold LN weights into adjacent matmuls; bf16-cast. Same on all cores."""
    L = cfg["L"]
    bf = ml_dtypes.bfloat16
    per_layer = []
    for l in range(L):
        w1, b1 = inputs["ln1_w"][l], inputs["ln1_b"][l]
        wq = np.asarray(inputs["attn_w"][l], np.float32)
        bq = np.asarray(inputs["attn_b"][l], np.float32)
        wq_eff = w1[:, None] * wq
        bq_eff = bq + b1 @ wq
        wq_eff = wq_eff.copy()
        bq_eff = bq_eff.copy()
        wq_eff[:, :D] *= 1.0 / np.sqrt(HD)
        bq_eff[:D] *= 1.0 / np.sqrt(HD)

        w2, b2 = inputs["ln2_w"][l], inputs["ln2_b"][l]
        wf1 = np.asarray(inputs["fc1_w"][l], np.float32)
        bf1 = np.asarray(inputs["fc1_b"][l], np.float32)
        wf1_eff = w2[:, None] * wf1
        bf1_eff = bf1 + b2 @ wf1

        bias_pack = np.concatenate([
            bq_eff, np.asarray(inputs["proj_b"][l], np.float32),
            bf1_eff, np.asarray(inputs["fc2_b"][l], np.float32)
        ]).astype(np.float32)
        assert bias_pack.shape[0] == 54 * P

        per_layer.append(dict(
            wqkv=np.ascontiguousarray(wq_eff).astype(bf),
            wproj=np.ascontiguousarray(
                np.asarray(inputs["proj_w"][l], np.float32)).astype(bf),
            wfc1=np.ascontiguousarray(wf1_eff).astype(bf),
            wfc2=np.ascontiguousarray(
                np.asarray(inputs["fc2_w"][l], np.float32)).astype(bf),
            bias=bias_pack,
        ))
    return per_layer


def host_prep(inputs, cfg):
    """Build the 8 per-core input maps (weights are NEFF constants)."""
    VSH = cfg["VSH"]
    B, T, V = cfg["B"], cfg["T"], cfg["V"]
    bf = ml_dtypes.bfloat16

    idx = np.asarray(inputs["idx"])
    wte = np.asarray(inputs["wte"], np.float32)
    wpe = np.asarray(inputs["wpe"], np.float32)

    x0 = wte[idx] + wpe[None, :T]                      # [B,T,D] f32

    # lm head: fold lnf into wte (tied); pad vocab to NCORES*VSH
    lnf_w = np.asarray(inputs["lnf_w"], np.float32)
    lnf_b = np.asarray(inputs["lnf_b"], np.float32)
    VP = NCORES * VSH
    wtet = np.zeros((D, VP), np.float32)
    wtet[:, :V] = (wte * lnf_w[None, :]).T
    bhead = np.zeros(VP, np.float32)
    bhead[:V] = wte @ lnf_b

    in_maps = []
    for c in range(NCORES):
        b, j = c // NSH, c % NSH
        x0t = np.ascontiguousarray(x0[b, j * TOK:(j + 1) * TOK].T,
                                   dtype=np.float32)
        # mask[k, q] = 1 if key k <= global query position
        qpos = j * TOK + np.arange(TOK)
        kpos = np.arange(NSH * TOK)
        mask = (kpos[:, None] <= qpos[None, :]).astype(bf)
        m = dict(x0t=x0t, mask01t=mask)
        m["wtet"] = np.ascontiguousarray(wtet[:, c * VSH:(c + 1) * VSH]).astype(bf)
        m["bhead"] = np.ascontiguousarray(bhead[c * VSH:(c + 1) * VSH])
        in_maps.append(m)
    return in_maps


def assemble(results, inputs, cfg):
    """Full logits [B,T,V] + mean CE loss from per-core logit shards."""
    B, T, V = cfg["B"], cfg["T"], cfg["V"]
    shards = [results[c]["logitst"] for c in range(NCORES)]
    lg = np.concatenate(shards, axis=0)[:V]            # [V, 2048]
    logits = np.ascontiguousarray(lg.T).reshape(B, T, V)

    targets = np.asarray(inputs["targets"]).reshape(-1)
    flat = logits.reshape(-1, V)
    n = flat.shape[0]
    lse = np.empty(n, np.float64)
    for i in range(0, n, 256):
        blk = flat[i:i + 256]
        m = blk.max(axis=1)
        lse[i:i + 256] = m + np.log(
            np.exp(blk - m[:, None], dtype=np.float32).sum(axis=1,
                                                           dtype=np.float64))
    valid = targets != -1
    tgt = np.maximum(targets, 0)
    nll = lse - flat[np.arange(n), tgt]
    loss = np.float32(np.where(valid, nll, 0.0).sum() / max(valid.sum(), 1))
    return logits, loss


_NC_CACHE = {}


def kernel(**inputs):
    cfg = FULL_CFG
    per_layer = fold_weights(inputs, cfg)
    import hashlib
    hsh = hashlib.blake2b(digest_size=16)
    for pl in per_layer:
        for k in ("wqkv", "wproj", "wfc1", "wfc2", "bias"):
            hsh.update(pl[k].tobytes())
    key = hsh.hexdigest()
    if _NC_CACHE.get("key") != key:
        _NC_CACHE["nc"] = build_nc(cfg, per_layer)
        _NC_CACHE["key"] = key
    nc = _NC_CACHE["nc"]
    in_maps = host_prep(inputs, cfg)
    res = bass_utils.run_bass_kernel_spmd(nc, in_maps,
                                          core_ids=list(range(NCORES)))
    return assemble(res.results, inputs, cfg)


# revision 17
# speedup vs baseline: 25938.2689x; 25938.2689x over previous
"""Trainium2 Bass kernel for a 4-layer GPT (B=2, T=1024, D=768, H=12, V=32000).

Sharding: 8 cores = 2 (batch) x 4 (token chunks of 256).  Each layer:
token-local qkv -> AllGather(K,V) within the 4-core batch group -> full
causal attention for own queries -> proj+residual -> MLP (token-local).
Final: 8-core AllGather of hidden states, vocab-sharded tied lm_head
(4096 padded-vocab rows per core).  Host: embedding gather, LN folding
into adjacent matmul weights, bf16 weight casting, logits assembly and
cross-entropy loss.

Self-contained: hardcodes all shapes from the problem spec.
"""

import sys

sys.path.insert(0, "/opt/trn_rl_repo")

import numpy as np
import ml_dtypes

import concourse.bass as bass
import concourse.mybir as mybir
import concourse.tile as tile
from concourse import bacc, bass_utils
from concourse.masks import make_identity

P = 128
D = 768
DO = D // P          # 6
H = 12
HD = 64
TOK = 256            # tokens per core
NSH = 4              # shards (cores) per batch group
KCH = NSH * TOK // P # 8 key chunks of 128
QKVO = 3 * D // P    # 18
F1O = 4 * D // P     # 24
NCORES = 8

FULL_CFG = dict(L=4, VSH=4096, B=2, T=1024, V=32000)

F32 = mybir.dt.float32
BF16 = mybir.dt.bfloat16
F16 = mybir.dt.float16


def build_nc(cfg, weights):
    """weights: list of per-layer dicts (wqkv/wproj/wfc1/wfc2 bf16, bias f32)
    identical on all cores -- embedded in the NEFF as Const tensors."""
    L = cfg["L"]
    VSH = cfg["VSH"]
    VO = VSH // P          # vocab chunks per core
    VSUB = VSH // 512      # streamed wte subtiles

    nc = bacc.Bacc("TRN2", target_bir_lowering=False, debug=False,
                   num_devices=NCORES)

    # ---- DRAM I/O ----
    x0t_d = nc.dram_tensor("x0t", [D, TOK], F32, kind="ExternalInput")
    mask_d = nc.dram_tensor("mask01t", [NSH * TOK, TOK], BF16,
                            kind="ExternalInput")
    w_d = []
    for l in range(L):
        w_d.append({k: nc.inline_tensor(weights[l][k], name=f"{k}_{l}")
                    for k in ("wqkv", "wproj", "wfc1", "wfc2", "bias")})
    wtet_d = nc.dram_tensor("wtet", [D, VSH], BF16, kind="ExternalInput")
    bhead_d = nc.dram_tensor("bhead", [VSH], F32, kind="ExternalInput")
    logits_d = nc.dram_tensor("logitst", [VSH, 2048], F16, kind="ExternalOutput")

    KBYTES = D * TOK          # elements in k^T part of the kv exchange buffer
    KVLEN = 2 * KBYTES        # k^T then v(token-major), flat

    with tile.TileContext(nc) as tc:
        with (
            tc.tile_pool(name="const", bufs=1) as cpool,
            tc.tile_pool(name="state", bufs=1) as spool,
            tc.tile_pool(name="acts", bufs=1) as apool,
            tc.tile_pool(name="wts", bufs=3) as wpool,
            tc.tile_pool(name="bias", bufs=2) as bpool,
            tc.tile_pool(name="tmp", bufs=3) as tpool,
            tc.tile_pool(name="pmask", bufs=2) as ppool,
            tc.tile_pool(name="mmps", bufs=4, space="PSUM") as mmps,
            tc.tile_pool(name="dram", bufs=1, space="DRAM") as dpool,
        ):
            # constants
            ones_f = cpool.tile([P, P], F32, name="ones_f")
            nc.gpsimd.memset(ones_f[:], 1.0)
            ones_b = cpool.tile([P, 1], BF16, name="ones_b")
            nc.gpsimd.memset(ones_b[:], 1.0)
            ident = cpool.tile([P, P], BF16, name="ident")
            make_identity(nc, ident[:])
            eps_t = cpool.tile([1, 1], F32, name="eps_t")
            nc.gpsimd.memset(eps_t[:], 1e-5)
            maskt = cpool.tile([P, KCH, TOK], BF16, name="maskt")
            nc.sync.dma_start(
                maskt[:], mask_d.ap().rearrange("(kc kp) q -> kp kc q", kp=P))

            # residual stream, f32, feature-major [feat%128, feat//128, tok]
            x = spool.tile([P, DO, TOK], F32, name="x")
            nc.sync.dma_start(
                x[:], x0t_d.ap().rearrange("(o p) t -> p o t", p=P))

            def layer_norm(x_in, out_tile):
                """out = (x - mean)/std featurewise (weights folded on host)."""
                sq = tpool.tile([P, DO, TOK], F32, name="sq", tag="sq", bufs=1)
                nc.vector.tensor_tensor(sq[:], x_in[:], x_in[:],
                                        mybir.AluOpType.mult)
                s_ps = mmps.tile([P, 512], F32, name="s_ps", tag="mm")
                q_ps = mmps.tile([P, 512], F32, name="q_ps", tag="mm")
                for ko in range(DO):
                    nc.tensor.matmul(s_ps[0:1, 0:TOK], ones_f[:, 0:1],
                                     x_in[:, ko, :],
                                     start=(ko == 0), stop=(ko == DO - 1))
                    nc.tensor.matmul(q_ps[0:1, 0:TOK], ones_f[:, 0:1],
                                     sq[:, ko, :],
                                     start=(ko == 0), stop=(ko == DO - 1))
                mr = tpool.tile([1, 2 * TOK], F32, name="mr", tag="mr", bufs=2)
                # mean into [0:TOK]
                nc.scalar.activation(mr[0:1, 0:TOK], s_ps[0:1, 0:TOK],
                                     mybir.ActivationFunctionType.Copy,
                                     scale=1.0 / D)
                # E[x^2]
                ex2 = tpool.tile([1, TOK], F32, name="ex2", tag="ex2", bufs=2)
                nc.scalar.activation(ex2[0:1, :], q_ps[0:1, 0:TOK],
                                     mybir.ActivationFunctionType.Copy,
                                     scale=1.0 / D)
                # var = ex2 - mean^2, via (mean * mean) then subtract
                var = tpool.tile([1, TOK], F32, name="var", tag="var", bufs=2)
                nc.vector.tensor_tensor(var[0:1, :], mr[0:1, 0:TOK],
                                        mr[0:1, 0:TOK], mybir.AluOpType.mult)
                nc.vector.tensor_tensor(var[0:1, :], ex2[0:1, :], var[0:1, :],
                                        mybir.AluOpType.subtract)
                sd = tpool.tile([1, TOK], F32, name="sd", tag="sd", bufs=2)
                nc.scalar.activation(sd[0:1, :], var[0:1, :],
                                     mybir.ActivationFunctionType.Sqrt,
                                     bias=eps_t[0:1, 0:1])
                nc.vector.reciprocal(mr[0:1, TOK:2 * TOK], sd[0:1, :])
                b_ps = mmps.tile([P, 512], F32, name="b_ps", tag="mm")
                nc.tensor.matmul(b_ps[:, 0:2 * TOK], ones_f[0:1, :],
                                 mr[0:1, :], start=True, stop=True)
                for ko in range(DO):
                    t1 = tpool.tile([P, TOK], F32, name="lnt", tag="lnt", bufs=3)
                    nc.vector.tensor_tensor(t1[:], x_in[:, ko, :],
                                            b_ps[:, 0:TOK],
                                            mybir.AluOpType.subtract)
                    nc.vector.tensor_tensor(out_tile[:, ko, :], t1[:],
                                            b_ps[:, TOK:2 * TOK],
                                            mybir.AluOpType.mult)

            for l in range(L):
                wd = w_d[l]
                bias_t = bpool.tile([P, 54], F32, name=f"bias_t{l}", tag="bias")
                nc.sync.dma_start(
                    bias_t[:], wd["bias"].ap().rearrange("(o p) -> p o", p=P))
                bq = bias_t[:, 0:QKVO]
                bp = bias_t[:, QKVO:QKVO + DO]
                b1 = bias_t[:, QKVO + DO:QKVO + DO + F1O]
                b2 = bias_t[:, QKVO + DO + F1O:54]

                # ---- LN1 + qkv ----
                h = apool.tile([P, DO, TOK], BF16, name="h", tag="h")
                layer_norm(x, h)
                qkvt = apool.tile([P, QKVO, TOK], BF16, name="qkvt", tag="qkvt")
                wq_view = wd["wqkv"].ap().rearrange("(ko kp) m -> kp ko m", kp=P)
                for ms in range(3):
                    wsub = wpool.tile([P, DO, D], BF16, name=f"wq{l}_{ms}",
                                      tag="wsub")
                    nc.sync.dma_start(wsub[:],
                                      wq_view[:, :, ms * D:(ms + 1) * D])
                    for mo in range(DO):
                        ps = mmps.tile([P, 512], F32, name="qkv_ps", tag="mm")
                        for ko in range(DO):
                            nc.tensor.matmul(
                                ps[:, 0:TOK],
                                wsub[:, ko, mo * P:(mo + 1) * P],
                                h[:, ko, :],
                                start=(ko == 0), stop=(ko == DO - 1))
                        gm = 6 * ms + mo
                        nc.scalar.activation(
                            qkvt[:, gm, :], ps[:, 0:TOK],
                            mybir.ActivationFunctionType.Identity,
                            bias=bq[:, gm:gm + 1])

                # ---- local v -> token-major ----
                vtok = apool.tile([P, 2, D], BF16, name="vtok", tag="vtok")
                with tc.tile_pool(name="vtps", bufs=2, space="PSUM") as vtps:
                    for o in range(DO):
                        for to in range(2):
                            tp_ps = vtps.tile([P, P], BF16, name="tp_ps")
                            nc.tensor.transpose(
                                tp_ps[:], qkvt[:, 12 + o, to * P:(to + 1) * P],
                                ident[:])
                            nc.scalar.activation(
                                vtok[:, to, o * P:(o + 1) * P], tp_ps[:],
                                mybir.ActivationFunctionType.Copy)

                # ---- kv exchange (AllGather within batch group) ----
                kv_in = dpool.tile([KVLEN], BF16, name=f"kv_in{l}",
                                   tag="kv_in", bufs=2)
                nc.sync.dma_start(
                    kv_in[0:KBYTES].rearrange("(o p t) -> p o t", p=P, t=TOK),
                    qkvt[:, 6:12, :])
                nc.sync.dma_start(
                    kv_in[KBYTES:KVLEN].rearrange("(to tp d) -> tp to d",
                                                  tp=P, d=D),
                    vtok[:])
                kv_g = dpool.tile([NSH, KVLEN], BF16, name=f"kv_g{l}",
                                  tag="kv_g", bufs=2)
                nc.gpsimd.collective_compute(
                    "AllGather", mybir.AluOpType.bypass,
                    replica_groups=[[0, 1, 2, 3], [4, 5, 6, 7]],
                    ins=[kv_in[:]], outs=[kv_g[:]])

                kt = apool.tile([P, DO, NSH, TOK], BF16, name="kt", tag="kt")
                vt = apool.tile([P, NSH, 2, D], BF16, name="vt", tag="vt")
                for si in range(NSH):
                    nc.sync.dma_start(
                        kt[:, :, si, :],
                        kv_g[si, 0:KBYTES].rearrange("(o p t) -> p o t",
                                                     p=P, t=TOK))
                    nc.sync.dma_start(
                        vt[:, si, :, :],
                        kv_g[si, KBYTES:KVLEN].rearrange("(to tp d) -> tp to d",
                                                         tp=P, d=D))

                # ---- attention ----
                yt = apool.tile([P, DO, TOK], BF16, name="yt", tag="yt")
                with tc.tile_pool(name="attnps", bufs=2, space="PSUM") as atps:
                  for hg in range(2):
                    # phase A: scores + exp + mask for a group of 6 heads
                    pms = {}
                    for hh in range(hg * H // 2, (hg + 1) * H // 2):
                        ko_h = hh // 2
                        p0 = HD * (hh % 2)
                        pm = ppool.tile([P, KCH, TOK], BF16,
                                        name=f"pm{hh}", tag="pm",
                                        bufs=H // 2)
                        for half in range(2):
                            s_ps = atps.tile([P, KCH // 2, TOK], F32,
                                             name="s_ps", tag="s_ps")
                            for kk in range(KCH // 2):
                                kc = half * (KCH // 2) + kk
                                si, th = kc // 2, kc % 2
                                nc.tensor.matmul(
                                    s_ps[:, kk, :],
                                    kt[p0:p0 + HD, ko_h, si,
                                       th * P:(th + 1) * P],
                                    qkvt[p0:p0 + HD, ko_h, :],
                                    start=True, stop=True)
                            hs = slice(half * (KCH // 2),
                                       (half + 1) * (KCH // 2))
                            nc.scalar.activation(
                                pm[:, hs, :], s_ps[:],
                                mybir.ActivationFunctionType.Exp)
                            nc.vector.tensor_tensor(pm[:, hs, :], pm[:, hs, :],
                                                    maskt[:, hs, :],
                                                    mybir.AluOpType.mult)
                        pms[hh] = pm
                    # phase B: per-head normalization + AV
                    for hh in range(hg * H // 2, (hg + 1) * H // 2):
                        ko_h = hh // 2
                        p0 = HD * (hh % 2)
                        pm = pms[hh]
                        sum_ps = mmps.tile([P, 512], F32, name="sum_ps", tag="mm")
                        for kc in range(KCH):
                            nc.tensor.matmul(sum_ps[0:1, 0:TOK], ones_b[:],
                                             pm[:, kc, :],
                                             start=(kc == 0),
                                             stop=(kc == KCH - 1))
                        rcp = tpool.tile([1, TOK], F32, name="rcp", tag="rcp",
                                         bufs=3)
                        nc.vector.reciprocal(rcp[0:1, :], sum_ps[0:1, 0:TOK])
                        rb_ps = mmps.tile([P, 512], F32, name="rb_ps", tag="mm")
                        nc.tensor.matmul(rb_ps[:, 0:TOK], ones_f[0:1, :],
                                         rcp[0:1, :], start=True, stop=True)
                        rb = tpool.tile([P, TOK], F32, name="rb", tag="rb",
                                        bufs=3)
                        nc.scalar.activation(
                            rb[:], rb_ps[:, 0:TOK],
                            mybir.ActivationFunctionType.Copy)
                        av_ps = mmps.tile([P, 512], F32, name="av_ps", tag="mm")
                        for kc in range(KCH):
                            nc.tensor.matmul(
                                av_ps[0:HD, 0:TOK],
                                vt[:, kc // 2, kc % 2, hh * HD:(hh + 1) * HD],
                                pm[:, kc, :],
                                start=(kc == 0), stop=(kc == KCH - 1))
                        nc.vector.tensor_tensor(
                            yt[p0:p0 + HD, ko_h, :], av_ps[0:HD, 0:TOK],
                            rb[0:HD, :], mybir.AluOpType.mult)

                # ---- proj + residual ----
                wp = wpool.tile([P, DO, D], BF16, name=f"wp{l}", tag="wsub")
                nc.sync.dma_start(
                    wp[:], wd["wproj"].ap().rearrange("(ko kp) m -> kp ko m",
                                                      kp=P))
                for mo in range(DO):
                    ps = mmps.tile([P, 512], F32, name="proj_ps", tag="mm")
                    for ko in range(DO):
                        nc.tensor.matmul(ps[:, 0:TOK],
                                         wp[:, ko, mo * P:(mo + 1) * P],
                                         yt[:, ko, :],
                                         start=(ko == 0), stop=(ko == DO - 1))
                    nc.vector.scalar_tensor_tensor(
                        x[:, mo, :], ps[:, 0:TOK], bp[:, mo:mo + 1],
                        x[:, mo, :],
                        mybir.AluOpType.add, mybir.AluOpType.add)

                # ---- LN2 + MLP ----
                h2 = apool.tile([P, DO, TOK], BF16, name="h2", tag="h")
                layer_norm(x, h2)
                g = apool.tile([P, F1O, TOK], BF16, name="g", tag="g")
                w1_view = wd["wfc1"].ap().rearrange("(ko kp) m -> kp ko m", kp=P)
                for ms in range(4):
                    wsub = wpool.tile([P, DO, D], BF16, name=f"w1{l}_{ms}",
                                      tag="wsub")
                    nc.sync.dma_start(wsub[:],
                                      w1_view[:, :, ms * D:(ms + 1) * D])
                    for mo in range(DO):
                        ps = mmps.tile([P, 512], F32, name="fc1_ps", tag="mm")
                        for ko in range(DO):
                            nc.tensor.matmul(
                                ps[:, 0:TOK],
                                wsub[:, ko, mo * P:(mo + 1) * P],
                                h2[:, ko, :],
                                start=(ko == 0), stop=(ko == DO - 1))
                        gm = 6 * ms + mo
                        nc.scalar.activation(
                            g[:, gm, :], ps[:, 0:TOK],
                            mybir.ActivationFunctionType.Gelu,
                            bias=b1[:, gm:gm + 1])

                w2_view = wd["wfc2"].ap().rearrange("(ko kp) m -> kp ko m", kp=P)
                w2subs = []
                for ks in range(4):
                    wsub = wpool.tile([P, DO, D], BF16, name=f"w2{l}_{ks}",
                                      tag="w2sub", bufs=4)
                    nc.sync.dma_start(
                        wsub[:],
                        w2_view[:, ks * DO:(ks + 1) * DO, :])
                    w2subs.append(wsub)
                with tc.tile_pool(name="fc2ps", bufs=1, space="PSUM") as f2ps:
                    ps = f2ps.tile([P, DO, TOK], F32, name="fc2_ps")
                    for mo in range(DO):
                        for ks in range(4):
                            for ko in range(DO):
                                nc.tensor.matmul(
                                    ps[:, mo, :],
                                    w2subs[ks][:, ko, mo * P:(mo + 1) * P],
                                    g[:, 6 * ks + ko, :],
                                    start=(ks == 0 and ko == 0),
                                    stop=(ks == 3 and ko == DO - 1))
                    for mo in range(DO):
                        nc.vector.scalar_tensor_tensor(
                            x[:, mo, :], ps[:, mo, :], b2[:, mo:mo + 1],
                            x[:, mo, :],
                            mybir.AluOpType.add, mybir.AluOpType.add)

            # ---- final LN + hidden-state AllGather ----
            xhat = apool.tile([P, DO, TOK], BF16, name="xhat", tag="h")
            layer_norm(x, xhat)
            ag2_in = dpool.tile([D * TOK], BF16, name="ag2_in")
            nc.sync.dma_start(
                ag2_in[:].rearrange("(o p t) -> p o t", p=P, t=TOK), xhat[:])
            xall_d = dpool.tile([NCORES, D * TOK], BF16, name="xall_d",
                                addr_space="Shared")
            nc.gpsimd.collective_compute(
                "AllGather", mybir.AluOpType.bypass,
                replica_groups=[[0, 1, 2, 3, 4, 5, 6, 7]],
                ins=[ag2_in[:]], outs=[xall_d[:]])
            xall = apool.tile([P, DO, NCORES, TOK], BF16, name="xall")
            for si in range(NCORES):
                nc.sync.dma_start(
                    xall[:, :, si, :],
                    xall_d[si, :].rearrange("(o p t) -> p o t", p=P, t=TOK))

            # ---- lm head ----
            bh_t = cpool.tile([P, VO], F32, name="bh_t")
            nc.sync.dma_start(
                bh_t[:], bhead_d.ap().rearrange("(o p) -> p o", p=P))
            wte_view = wtet_d.ap().rearrange("(ko kp) v -> kp ko v", kp=P)
            lg_view = logits_d.ap().rearrange("(vc vp) t -> vp vc t", vp=P)
            for vs in range(VSUB):
                wt = wpool.tile([P, DO, 512], BF16, name=f"wt{vs}", tag="wsub")
                nc.sync.dma_start(wt[:],
                                  wte_view[:, :, vs * 512:(vs + 1) * 512])
                for vloc in range(4):
                    vc = 4 * vs + vloc
                    for tcb in range(4):
                        ps = mmps.tile([P, 512], F32, name="hd_ps", tag="mm")
                        for ko in range(DO):
                            nc.tensor.matmul(
                                ps[:],
                                wt[:, ko, vloc * P:(vloc + 1) * P],
                                xall[:, ko, 2 * tcb:2 * tcb + 2, :],
                                start=(ko == 0), stop=(ko == DO - 1))
                        lsb = tpool.tile([P, 512], F16, name="lsb", tag="lsb",
                                         bufs=4)
                        nc.scalar.activation(
                            lsb[:], ps[:],
                            mybir.ActivationFunctionType.Identity,
                            bias=bh_t[:, vc:vc + 1])
                        nc.sync.dma_start(
                            lg_view[:, vc, tcb * 512:(tcb + 1) * 512], lsb[:])

    nc.compile()
    return nc


# ----------------------------------------------------------------------
# Host-side preparation
# ----------------------------------------------------------------------

def fold_weights(inputs, cfg):
    """Fold LN weights into adjacent matmuls; bf16-cast. Same on all cores."""
    L = cfg["L"]
    bf = ml_dtypes.bfloat16
    inputs = {k: np.asarray(v) for k, v in inputs.items()}
    per_layer = []
    for l in range(L):
        w1, b1 = inputs["ln1_w"][l], inputs["ln1_b"][l]
        wq = np.asarray(inputs["attn_w"][l], np.float32)
        bq = np.asarray(inputs["attn_b"][l], np.float32)
        wq_eff = w1[:, None] * wq
        bq_eff = bq + b1 @ wq
        wq_eff = wq_eff.copy()
        bq_eff = bq_eff.copy()
        wq_eff[:, :D] *= 1.0 / np.sqrt(HD)
        bq_eff[:D] *= 1.0 / np.sqrt(HD)

        w2, b2 = inputs["ln2_w"][l], inputs["ln2_b"][l]
        wf1 = np.asarray(inputs["fc1_w"][l], np.float32)
        bf1 = np.asarray(inputs["fc1_b"][l], np.float32)
        wf1_eff = w2[:, None] * wf1
        bf1_eff = bf1 + b2 @ wf1

        bias_pack = np.concatenate([
            bq_eff, np.asarray(inputs["proj_b"][l], np.float32),
            bf1_eff, np.asarray(inputs["fc2_b"][l], np.float32)
        ]).astype(np.float32)
        assert bias_pack.shape[0] == 54 * P

        per_layer.append(dict(
            wqkv=np.ascontiguousarray(wq_eff).astype(bf),
            wproj=np.ascontiguousarray(
                np.asarray(inputs["proj_w"][l], np.float32)).astype(bf),
            wfc1=np.ascontiguousarray(wf1_eff).astype(bf),
            wfc2=np.ascontiguousarray(
                np.asarray(inputs["fc2_w"][l], np.float32)).astype(bf),
            bias=bias_pack,
        ))
    return per_layer


def host_prep(inputs, cfg):
    """Build the 8 per-core input maps (weights are NEFF constants)."""
    VSH = cfg["VSH"]
    B, T, V = cfg["B"], cfg["T"], cfg["V"]
    bf = ml_dtypes.bfloat16

    idx = np.asarray(inputs["idx"])
    wte = np.asarray(inputs["wte"], np.float32)
    wpe = np.asarray(inputs["wpe"], np.float32)

    x0 = wte[idx] + wpe[None, :T]                      # [B,T,D] f32

    # lm head: fold lnf into wte (tied); pad vocab to NCORES*VSH
    lnf_w = np.asarray(inputs["lnf_w"], np.float32)
    lnf_b = np.asarray(inputs["lnf_b"], np.float32)
    VP = NCORES * VSH
    wtet = np.zeros((D, VP), np.float32)
    wtet[:, :V] = (wte * lnf_w[None, :]).T
    bhead = np.zeros(VP, np.float32)
    bhead[:V] = wte @ lnf_b

    in_maps = []
    for c in range(NCORES):
        b, j = c // NSH, c % NSH
        x0t = np.ascontiguousarray(x0[b, j * TOK:(j + 1) * TOK].T,
                                   dtype=np.float32)
        # mask[k, q] = 1 if key k <= global query position
        qpos = j * TOK + np.arange(TOK)
        kpos = np.arange(NSH * TOK)
        mask = (kpos[:, None] <= qpos[None, :]).astype(bf)
        m = dict(x0t=x0t, mask01t=mask)
        m["wtet"] = np.ascontiguousarray(wtet[:, c * VSH:(c + 1) * VSH]).astype(bf)
        m["bhead"] = np.ascontiguousarray(bhead[c * VSH:(c + 1) * VSH])
        in_maps.append(m)
    return in_maps


def assemble(results, inputs, cfg):
    """Full logits [B,T,V] + mean CE loss from per-core logit shards."""
    B, T, V = cfg["B"], cfg["T"], cfg["V"]
    shards = [np.asarray(results[c]["logitst"], np.float32)
              for c in range(NCORES)]
    lg = np.concatenate(shards, axis=0)[:V]            # [V, 2048]
    logits = np.ascontiguousarray(lg.T).reshape(B, T, V)

    targets = np.asarray(inputs["targets"]).reshape(-1)
    flat = logits.reshape(-1, V)
    n = flat.shape[0]
    lse = np.empty(n, np.float64)
    for i in range(0, n, 256):
        blk = flat[i:i + 256]
        m = blk.max(axis=1)
        lse[i:i + 256] = m + np.log(
            np.exp(blk - m[:, None], dtype=np.float32).sum(axis=1,
                                                           dtype=np.float64))
    valid = targets != -1
    tgt = np.maximum(targets, 0)
    nll = lse - flat[np.arange(n), tgt]
    loss = np.float32(np.where(valid, nll, 0.0).sum() / max(valid.sum(), 1))
    return logits, loss


_NC_CACHE = {}


def kernel(**inputs):
    cfg = FULL_CFG
    inputs = {k: np.asarray(v) for k, v in inputs.items()}
    per_layer = fold_weights(inputs, cfg)
    import hashlib
    hsh = hashlib.blake2b(digest_size=16)
    for pl in per_layer:
        for k in ("wqkv", "wproj", "wfc1", "wfc2", "bias"):
            hsh.update(pl[k].tobytes())
    key = hsh.hexdigest()
    if _NC_CACHE.get("key") != key:
        _NC_CACHE["nc"] = build_nc(cfg, per_layer)
        _NC_CACHE["key"] = key
    nc = _NC_CACHE["nc"]
    in_maps = host_prep(inputs, cfg)
    res = bass_utils.run_bass_kernel_spmd(nc, in_maps,
                                          core_ids=list(range(NCORES)))
    return assemble(res.results, inputs, cfg)


# revision 20
# speedup vs baseline: 27920.7061x; 1.0764x over previous
"""Trainium2 Bass kernel for a 4-layer GPT (B=2, T=1024, D=768, H=12, V=32000).

Sharding: 8 cores = 2 (batch) x 4 (token chunks of 256).  Each layer:
token-local qkv -> AllGather(K,V) within the 4-core batch group -> full
causal attention for own queries -> proj+residual -> MLP (token-local).
Final: 8-core AllGather of hidden states, vocab-sharded tied lm_head
(4096 padded-vocab rows per core).  Host: embedding gather, LN folding
into adjacent matmul weights, bf16 weight casting, logits assembly and
cross-entropy loss.

Self-contained: hardcodes all shapes from the problem spec.
"""

import sys

sys.path.insert(0, "/opt/trn_rl_repo")

import numpy as np
import ml_dtypes

import concourse.bass as bass
import concourse.mybir as mybir
import concourse.tile as tile
from concourse import bacc, bass_utils
from concourse.masks import make_identity

P = 128
D = 768
DO = D // P          # 6
H = 12
HD = 64
TOK = 256            # tokens per core
NSH = 4              # shards (cores) per batch group
KCH = NSH * TOK // P # 8 key chunks of 128
QKVO = 3 * D // P    # 18
F1O = 4 * D // P     # 24
NCORES = 8

FULL_CFG = dict(L=4, VSH=4096, B=2, T=1024, V=32000)

F32 = mybir.dt.float32
BF16 = mybir.dt.bfloat16
F16 = mybir.dt.float16


def build_nc(cfg, weights, weights_head):
    """weights: list of per-layer dicts (wqkv/wproj/wfc1/wfc2 bf16, bias f32)
    identical on all cores -- embedded in the NEFF as Const tensors."""
    L = cfg["L"]
    VSH = cfg["VSH"]
    VO = VSH // P          # vocab chunks per core
    VSUB = VSH // 512      # streamed wte subtiles

    nc = bacc.Bacc("TRN2", target_bir_lowering=False, debug=False,
                   num_devices=NCORES)

    # ---- DRAM I/O ----
    x0t_d = nc.dram_tensor("x0t", [D, TOK], F32, kind="ExternalInput")
    mask_d = nc.dram_tensor("mask01t", [NSH * TOK, TOK], BF16,
                            kind="ExternalInput")
    w_d = []
    for l in range(L):
        w_d.append({k: nc.inline_tensor(weights[l][k], name=f"{k}_{l}")
                    for k in ("wqkv", "wproj", "wfc1", "wfc2", "bias")})
    VP = NCORES * VSH
    wtet_d = nc.inline_tensor(weights_head["wtet"], name="wtet")
    logits_d = nc.dram_tensor("logitst", [TOK, VP], F16, kind="ExternalOutput")

    KBYTES = D * TOK          # elements in k^T part of the kv exchange buffer
    KVLEN = 2 * KBYTES        # k^T then v(token-major), flat

    with tile.TileContext(nc) as tc:
        with (
            tc.tile_pool(name="const", bufs=1) as cpool,
            tc.tile_pool(name="state", bufs=1) as spool,
            tc.tile_pool(name="acts", bufs=1) as apool,
            tc.tile_pool(name="wts", bufs=3) as wpool,
            tc.tile_pool(name="bias", bufs=2) as bpool,
            tc.tile_pool(name="tmp", bufs=3) as tpool,
            tc.tile_pool(name="pmask", bufs=2) as ppool,
            tc.tile_pool(name="mmps", bufs=4, space="PSUM") as mmps,
            tc.tile_pool(name="dram", bufs=1, space="DRAM") as dpool,
        ):
            # constants
            ones_f = cpool.tile([P, P], F32, name="ones_f")
            nc.gpsimd.memset(ones_f[:], 1.0)
            ones_b = cpool.tile([P, 1], BF16, name="ones_b")
            nc.gpsimd.memset(ones_b[:], 1.0)
            ident = cpool.tile([P, P], BF16, name="ident")
            make_identity(nc, ident[:])
            eps_t = cpool.tile([1, 1], F32, name="eps_t")
            nc.gpsimd.memset(eps_t[:], 1e-5)
            maskt = cpool.tile([P, KCH, TOK], BF16, name="maskt")
            nc.sync.dma_start(
                maskt[:], mask_d.ap().rearrange("(kc kp) q -> kp kc q", kp=P))

            # residual stream, f32, feature-major [feat%128, feat//128, tok]
            x = spool.tile([P, DO, TOK], F32, name="x")
            nc.sync.dma_start(
                x[:], x0t_d.ap().rearrange("(o p) t -> p o t", p=P))

            def layer_norm(x_in, out_tile):
                """out = (x - mean)/std featurewise (weights folded on host)."""
                sq = tpool.tile([P, DO, TOK], F32, name="sq", tag="sq", bufs=1)
                nc.vector.tensor_tensor(sq[:], x_in[:], x_in[:],
                                        mybir.AluOpType.mult)
                s_ps = mmps.tile([P, 512], F32, name="s_ps", tag="mm")
                q_ps = mmps.tile([P, 512], F32, name="q_ps", tag="mm")
                for ko in range(DO):
                    nc.tensor.matmul(s_ps[0:1, 0:TOK], ones_f[:, 0:1],
                                     x_in[:, ko, :],
                                     start=(ko == 0), stop=(ko == DO - 1))
                    nc.tensor.matmul(q_ps[0:1, 0:TOK], ones_f[:, 0:1],
                                     sq[:, ko, :],
                                     start=(ko == 0), stop=(ko == DO - 1))
                mr = tpool.tile([1, 2 * TOK], F32, name="mr", tag="mr", bufs=2)
                # mean into [0:TOK]
                nc.scalar.activation(mr[0:1, 0:TOK], s_ps[0:1, 0:TOK],
                                     mybir.ActivationFunctionType.Copy,
                                     scale=1.0 / D)
                # E[x^2]
                ex2 = tpool.tile([1, TOK], F32, name="ex2", tag="ex2", bufs=2)
                nc.scalar.activation(ex2[0:1, :], q_ps[0:1, 0:TOK],
                                     mybir.ActivationFunctionType.Copy,
                                     scale=1.0 / D)
                # var = ex2 - mean^2, via (mean * mean) then subtract
                var = tpool.tile([1, TOK], F32, name="var", tag="var", bufs=2)
                nc.vector.tensor_tensor(var[0:1, :], mr[0:1, 0:TOK],
                                        mr[0:1, 0:TOK], mybir.AluOpType.mult)
                nc.vector.tensor_tensor(var[0:1, :], ex2[0:1, :], var[0:1, :],
                                        mybir.AluOpType.subtract)
                sd = tpool.tile([1, TOK], F32, name="sd", tag="sd", bufs=2)
                nc.scalar.activation(sd[0:1, :], var[0:1, :],
                                     mybir.ActivationFunctionType.Sqrt,
                                     bias=eps_t[0:1, 0:1])
                nc.vector.reciprocal(mr[0:1, TOK:2 * TOK], sd[0:1, :])
                b_ps = mmps.tile([P, 512], F32, name="b_ps", tag="mm")
                nc.tensor.matmul(b_ps[:, 0:2 * TOK], ones_f[0:1, :],
                                 mr[0:1, :], start=True, stop=True)
                for ko in range(DO):
                    t1 = tpool.tile([P, TOK], F32, name="lnt", tag="lnt", bufs=3)
                    nc.vector.tensor_tensor(t1[:], x_in[:, ko, :],
                                            b_ps[:, 0:TOK],
                                            mybir.AluOpType.subtract)
                    nc.vector.tensor_tensor(out_tile[:, ko, :], t1[:],
                                            b_ps[:, TOK:2 * TOK],
                                            mybir.AluOpType.mult)

            for l in range(L):
                wd = w_d[l]
                bias_t = bpool.tile([P, 54], F32, name=f"bias_t{l}", tag="bias")
                nc.sync.dma_start(
                    bias_t[:], wd["bias"].ap().rearrange("(o p) -> p o", p=P))
                bq = bias_t[:, 0:QKVO]
                bp = bias_t[:, QKVO:QKVO + DO]
                b1 = bias_t[:, QKVO + DO:QKVO + DO + F1O]
                b2 = bias_t[:, QKVO + DO + F1O:54]

                # ---- LN1 + qkv ----
                h = apool.tile([P, DO, TOK], BF16, name="h", tag="h")
                layer_norm(x, h)
                qkvt = apool.tile([P, QKVO, TOK], BF16, name="qkvt", tag="qkvt")
                wq_view = wd["wqkv"].ap().rearrange("(ko kp) m -> kp ko m", kp=P)

                def qkv_sub(ms):
                    wsub = wpool.tile([P, DO, D], BF16, name=f"wq{l}_{ms}",
                                      tag="wsub")
                    nc.sync.dma_start(wsub[:],
                                      wq_view[:, :, ms * D:(ms + 1) * D])
                    for mo in range(DO):
                        ps = mmps.tile([P, 512], F32, name="qkv_ps", tag="mm")
                        for ko in range(DO):
                            nc.tensor.matmul(
                                ps[:, 0:TOK],
                                wsub[:, ko, mo * P:(mo + 1) * P],
                                h[:, ko, :],
                                start=(ko == 0), stop=(ko == DO - 1))
                        gm = 6 * ms + mo
                        nc.scalar.activation(
                            qkvt[:, gm, :], ps[:, 0:TOK],
                            mybir.ActivationFunctionType.Identity,
                            bias=bq[:, gm:gm + 1])

                # k first so its AllGather overlaps v/q compute; scores need
                # only K, so the V AllGather overlaps the S/exp phase too.
                qkv_sub(1)
                k_in = dpool.tile([KBYTES], BF16, name=f"k_in{l}",
                                  tag="k_in", bufs=2)
                nc.sync.dma_start(
                    k_in[:].rearrange("(o p t) -> p o t", p=P, t=TOK),
                    qkvt[:, 6:12, :])
                k_g = dpool.tile([NSH, KBYTES], BF16, name=f"k_g{l}",
                                 tag="k_g", bufs=2)
                nc.gpsimd.collective_compute(
                    "AllGather", mybir.AluOpType.bypass,
                    replica_groups=[[0, 1, 2, 3], [4, 5, 6, 7]],
                    ins=[k_in[:]], outs=[k_g[:]])

                qkv_sub(2)
                # ---- local v -> token-major ----
                vtok = apool.tile([P, 2, D], BF16, name="vtok", tag="vtok")
                with tc.tile_pool(name="vtps", bufs=2, space="PSUM") as vtps:
                    for o in range(DO):
                        for to in range(2):
                            tp_ps = vtps.tile([P, P], BF16, name="tp_ps")
                            nc.tensor.transpose(
                                tp_ps[:], qkvt[:, 12 + o, to * P:(to + 1) * P],
                                ident[:])
                            nc.scalar.activation(
                                vtok[:, to, o * P:(o + 1) * P], tp_ps[:],
                                mybir.ActivationFunctionType.Copy)
                v_in = dpool.tile([KBYTES], BF16, name=f"v_in{l}",
                                  tag="v_in", bufs=2)
                nc.sync.dma_start(
                    v_in[:].rearrange("(to tp d) -> tp to d", tp=P, d=D),
                    vtok[:])
                v_g = dpool.tile([NSH, KBYTES], BF16, name=f"v_g{l}",
                                 tag="v_g", bufs=2)
                nc.gpsimd.collective_compute(
                    "AllGather", mybir.AluOpType.bypass,
                    replica_groups=[[0, 1, 2, 3], [4, 5, 6, 7]],
                    ins=[v_in[:]], outs=[v_g[:]])

                qkv_sub(0)

                kt = apool.tile([P, DO, NSH, TOK], BF16, name="kt", tag="kt")
                vt = apool.tile([P, NSH, 2, D], BF16, name="vt", tag="vt")
                for si in range(NSH):
                    nc.sync.dma_start(
                        kt[:, :, si, :],
                        k_g[si, :].rearrange("(o p t) -> p o t",
                                             p=P, t=TOK))
                    nc.sync.dma_start(
                        vt[:, si, :, :],
                        v_g[si, :].rearrange("(to tp d) -> tp to d",
                                             tp=P, d=D))

                # ---- attention ----
                yt = apool.tile([P, DO, TOK], BF16, name="yt", tag="yt")
                with tc.tile_pool(name="attnps", bufs=2, space="PSUM") as atps:
                  for hg in range(2):
                    # phase A: scores + exp + mask for a group of 6 heads
                    pms = {}
                    for hh in range(hg * H // 2, (hg + 1) * H // 2):
                        ko_h = hh // 2
                        p0 = HD * (hh % 2)
                        pm = ppool.tile([P, KCH, TOK], BF16,
                                        name=f"pm{hh}", tag="pm",
                                        bufs=H // 2)
                        for half in range(2):
                            s_ps = atps.tile([P, KCH // 2, TOK], F32,
                                             name="s_ps", tag="s_ps")
                            for kk in range(KCH // 2):
                                kc = half * (KCH // 2) + kk
                                si, th = kc // 2, kc % 2
                                nc.tensor.matmul(
                                    s_ps[:, kk, :],
                                    kt[p0:p0 + HD, ko_h, si,
                                       th * P:(th + 1) * P],
                                    qkvt[p0:p0 + HD, ko_h, :],
                                    start=True, stop=True)
                            hs = slice(half * (KCH // 2),
                                       (half + 1) * (KCH // 2))
                            nc.scalar.activation(
                                pm[:, hs, :], s_ps[:],
                                mybir.ActivationFunctionType.Exp)
                            nc.vector.tensor_tensor(pm[:, hs, :], pm[:, hs, :],
                                                    maskt[:, hs, :],
                                                    mybir.AluOpType.mult)
                        pms[hh] = pm
                    # phase B: per-head normalization + AV
                    for hh in range(hg * H // 2, (hg + 1) * H // 2):
                        ko_h = hh // 2
                        p0 = HD * (hh % 2)
                        pm = pms[hh]
                        sum_ps = mmps.tile([P, 512], F32, name="sum_ps", tag="mm")
                        for kc in range(KCH):
                            nc.tensor.matmul(sum_ps[0:1, 0:TOK], ones_b[:],
                                             pm[:, kc, :],
                                             start=(kc == 0),
                                             stop=(kc == KCH - 1))
                        rcp = tpool.tile([1, TOK], F32, name="rcp", tag="rcp",
                                         bufs=3)
                        nc.vector.reciprocal(rcp[0:1, :], sum_ps[0:1, 0:TOK])
                        rb_ps = mmps.tile([P, 512], F32, name="rb_ps", tag="mm")
                        nc.tensor.matmul(rb_ps[:, 0:TOK], ones_f[0:1, :],
                                         rcp[0:1, :], start=True, stop=True)
                        rb = tpool.tile([P, TOK], F32, name="rb", tag="rb",
                                        bufs=3)
                        nc.scalar.activation(
                            rb[:], rb_ps[:, 0:TOK],
                            mybir.ActivationFunctionType.Copy)
                        av_ps = mmps.tile([P, 512], F32, name="av_ps", tag="mm")
                        for kc in range(KCH):
                            nc.tensor.matmul(
                                av_ps[0:HD, 0:TOK],
                                vt[:, kc // 2, kc % 2, hh * HD:(hh + 1) * HD],
                                pm[:, kc, :],
                                start=(kc == 0), stop=(kc == KCH - 1))
                        nc.vector.tensor_tensor(
                            yt[p0:p0 + HD, ko_h, :], av_ps[0:HD, 0:TOK],
                            rb[0:HD, :], mybir.AluOpType.mult)

                # ---- proj + residual ----
                wp = wpool.tile([P, DO, D], BF16, name=f"wp{l}", tag="wsub")
                nc.sync.dma_start(
                    wp[:], wd["wproj"].ap().rearrange("(ko kp) m -> kp ko m",
                                                      kp=P))
                for mo in range(DO):
                    ps = mmps.tile([P, 512], F32, name="proj_ps", tag="mm")
                    for ko in range(DO):
                        nc.tensor.matmul(ps[:, 0:TOK],
                                         wp[:, ko, mo * P:(mo + 1) * P],
                                         yt[:, ko, :],
                                         start=(ko == 0), stop=(ko == DO - 1))
                    nc.vector.scalar_tensor_tensor(
                        x[:, mo, :], ps[:, 0:TOK], bp[:, mo:mo + 1],
                        x[:, mo, :],
                        mybir.AluOpType.add, mybir.AluOpType.add)

                # ---- LN2 + MLP ----
                h2 = apool.tile([P, DO, TOK], BF16, name="h2", tag="h")
                layer_norm(x, h2)
                g = apool.tile([P, F1O, TOK], BF16, name="g", tag="g")
                w1_view = wd["wfc1"].ap().rearrange("(ko kp) m -> kp ko m", kp=P)
                for ms in range(4):
                    wsub = wpool.tile([P, DO, D], BF16, name=f"w1{l}_{ms}",
                                      tag="wsub")
                    nc.sync.dma_start(wsub[:],
                                      w1_view[:, :, ms * D:(ms + 1) * D])
                    for mo in range(DO):
                        ps = mmps.tile([P, 512], F32, name="fc1_ps", tag="mm")
                        for ko in range(DO):
                            nc.tensor.matmul(
                                ps[:, 0:TOK],
                                wsub[:, ko, mo * P:(mo + 1) * P],
                                h2[:, ko, :],
                                start=(ko == 0), stop=(ko == DO - 1))
                        gm = 6 * ms + mo
                        nc.scalar.activation(
                            g[:, gm, :], ps[:, 0:TOK],
                            mybir.ActivationFunctionType.Gelu,
                            bias=b1[:, gm:gm + 1])

                w2_view = wd["wfc2"].ap().rearrange("(ko kp) m -> kp ko m", kp=P)
                w2subs = []
                for ks in range(4):
                    wsub = wpool.tile([P, DO, D], BF16, name=f"w2{l}_{ks}",
                                      tag="w2sub", bufs=4)
                    nc.sync.dma_start(
                        wsub[:],
                        w2_view[:, ks * DO:(ks + 1) * DO, :])
                    w2subs.append(wsub)
                with tc.tile_pool(name="fc2ps", bufs=1, space="PSUM") as f2ps:
                    ps = f2ps.tile([P, DO, TOK], F32, name="fc2_ps")
                    for mo in range(DO):
                        for ks in range(4):
                            for ko in range(DO):
                                nc.tensor.matmul(
                                    ps[:, mo, :],
                                    w2subs[ks][:, ko, mo * P:(mo + 1) * P],
                                    g[:, 6 * ks + ko, :],
                                    start=(ks == 0 and ko == 0),
                                    stop=(ks == 3 and ko == DO - 1))
                    for mo in range(DO):
                        nc.vector.scalar_tensor_tensor(
                            x[:, mo, :], ps[:, mo, :], b2[:, mo:mo + 1],
                            x[:, mo, :],
                            mybir.AluOpType.add, mybir.AluOpType.add)

            # ---- final LN + token-sharded lm head (no collective) ----
            # logits token-major: lhsT = xhat block (tokens on the free dim
            # become output partitions), rhs = streamed wte (N=512).
            xhat = apool.tile([P, DO, TOK], BF16, name="xhat", tag="h")
            layer_norm(x, xhat)
            wte_view = wtet_d.ap().rearrange("(ko kp) v -> kp ko v", kp=P)
            for vs in range(VP // 512):
                wt = wpool.tile([P, DO, 512], BF16, name=f"wt{vs}", tag="wsub")
                nc.sync.dma_start(wt[:],
                                  wte_view[:, :, vs * 512:(vs + 1) * 512])
                for tch in range(TOK // P):
                    ps = mmps.tile([P, 512], F32, name="hd_ps", tag="mm")
                    for ko in range(DO):
                        nc.tensor.matmul(
                            ps[:],
                            xhat[:, ko, tch * P:(tch + 1) * P],
                            wt[:, ko, :],
                            start=(ko == 0), stop=(ko == DO - 1))
                    lsb = tpool.tile([P, 512], F16, name="lsb", tag="lsb",
                                     bufs=4)
                    nc.scalar.activation(
                        lsb[:], ps[:],
                        mybir.ActivationFunctionType.Copy)
                    nc.sync.dma_start(
                        logits_d.ap()[tch * P:(tch + 1) * P,
                                      vs * 512:(vs + 1) * 512], lsb[:])

    nc.compile()
    return nc


# ----------------------------------------------------------------------
# Host-side preparation
# ----------------------------------------------------------------------

def fold_weights(inputs, cfg):
    """Fold LN weights into adjacent matmuls; bf16-cast. Same on all cores."""
    L = cfg["L"]
    bf = ml_dtypes.bfloat16
    inputs = {k: np.asarray(v) for k, v in inputs.items()}
    per_layer = []
    for l in range(L):
        w1, b1 = inputs["ln1_w"][l], inputs["ln1_b"][l]
        wq = np.asarray(inputs["attn_w"][l], np.float32)
        bq = np.asarray(inputs["attn_b"][l], np.float32)
        wq_eff = w1[:, None] * wq
        bq_eff = bq + b1 @ wq
        wq_eff = wq_eff.copy()
        bq_eff = bq_eff.copy()
        wq_eff[:, :D] *= 1.0 / np.sqrt(HD)
        bq_eff[:D] *= 1.0 / np.sqrt(HD)

        w2, b2 = inputs["ln2_w"][l], inputs["ln2_b"][l]
        wf1 = np.asarray(inputs["fc1_w"][l], np.float32)
        bf1 = np.asarray(inputs["fc1_b"][l], np.float32)
        wf1_eff = w2[:, None] * wf1
        bf1_eff = bf1 + b2 @ wf1

        bias_pack = np.concatenate([
            bq_eff, np.asarray(inputs["proj_b"][l], np.float32),
            bf1_eff, np.asarray(inputs["fc2_b"][l], np.float32)
        ]).astype(np.float32)
        assert bias_pack.shape[0] == 54 * P

        per_layer.append(dict(
            wqkv=np.ascontiguousarray(wq_eff).astype(bf),
            wproj=np.ascontiguousarray(
                np.asarray(inputs["proj_w"][l], np.float32)).astype(bf),
            wfc1=np.ascontiguousarray(wf1_eff).astype(bf),
            wfc2=np.ascontiguousarray(
                np.asarray(inputs["fc2_w"][l], np.float32)).astype(bf),
            bias=bias_pack,
        ))

    VSH = cfg["VSH"]
    V = cfg["V"]
    VP = NCORES * VSH
    wte = np.asarray(inputs["wte"], np.float32)
    lnf_w = np.asarray(inputs["lnf_w"], np.float32)
    lnf_b = np.asarray(inputs["lnf_b"], np.float32)
    wtet = np.zeros((D, VP), np.float32)
    wtet[:, :V] = (wte * lnf_w[None, :]).T
    bhead = np.zeros(VP, np.float32)
    bhead[:V] = wte @ lnf_b
    head = dict(wtet=np.ascontiguousarray(wtet).astype(bf),
                bhead=bhead.astype(np.float32))
    return per_layer, head


def host_prep(inputs, cfg):
    """Build the 8 per-core input maps (weights are NEFF constants)."""
    VSH = cfg["VSH"]
    B, T, V = cfg["B"], cfg["T"], cfg["V"]
    bf = ml_dtypes.bfloat16

    idx = np.asarray(inputs["idx"])
    wte = np.asarray(inputs["wte"], np.float32)
    wpe = np.asarray(inputs["wpe"], np.float32)

    x0 = wte[idx] + wpe[None, :T]                      # [B,T,D] f32

    in_maps = []
    for c in range(NCORES):
        b, j = c // NSH, c % NSH
        x0t = np.ascontiguousarray(x0[b, j * TOK:(j + 1) * TOK].T,
                                   dtype=np.float32)
        # mask[k, q] = 1 if key k <= global query position
        qpos = j * TOK + np.arange(TOK)
        kpos = np.arange(NSH * TOK)
        mask = (kpos[:, None] <= qpos[None, :]).astype(bf)
        in_maps.append(dict(x0t=x0t, mask01t=mask))
    return in_maps


def assemble(results, inputs, cfg):
    """Full logits [B,T,V] + mean CE loss from per-core logit shards."""
    B, T, V = cfg["B"], cfg["T"], cfg["V"]
    wte = np.asarray(inputs["wte"], np.float32)
    lnf_b = np.asarray(inputs["lnf_b"], np.float32)
    bhead = wte @ lnf_b                                # [V]
    shards = [np.asarray(results[c]["logitst"], np.float32)
              for c in range(NCORES)]
    lg = np.concatenate(shards, axis=0)[:, :V]         # [2048, V] token-major
    lg += bhead[None, :]
    logits = lg.reshape(B, T, V)

    targets = np.asarray(inputs["targets"]).reshape(-1)
    flat = logits.reshape(-1, V)
    n = flat.shape[0]
    lse = np.empty(n, np.float64)
    for i in range(0, n, 256):
        blk = flat[i:i + 256]
        m = blk.max(axis=1)
        lse[i:i + 256] = m + np.log(
            np.exp(blk - m[:, None], dtype=np.float32).sum(axis=1,
                                                           dtype=np.float64))
    valid = targets != -1
    tgt = np.maximum(targets, 0)
    nll = lse - flat[np.arange(n), tgt]
    loss = np.float32(np.where(valid, nll, 0.0).sum() / max(valid.sum(), 1))
    return logits, loss


_NC_CACHE = {}


def kernel(**inputs):
    cfg = FULL_CFG
    inputs = {k: np.asarray(v) for k, v in inputs.items()}
    per_layer, head = fold_weights(inputs, cfg)
    import hashlib
    hsh = hashlib.blake2b(digest_size=16)
    for pl in per_layer:
        for k in ("wqkv", "wproj", "wfc1", "wfc2", "bias"):
            hsh.update(pl[k].tobytes())
    hsh.update(head["wtet"].tobytes())
    hsh.update(head["bhead"].tobytes())
    key = hsh.hexdigest()
    if _NC_CACHE.get("key") != key:
        _NC_CACHE["nc"] = build_nc(cfg, per_layer, head)
        _NC_CACHE["key"] = key
    nc = _NC_CACHE["nc"]
    in_maps = host_prep(inputs, cfg)
    res = bass_utils.run_bass_kernel_spmd(nc, in_maps,
                                          core_ids=list(range(NCORES)))
    return assemble(res.results, inputs, cfg)
